# revision 3
# baseline (speedup 1.0000x reference)
"""Trainium2 Bass kernel for CrossAttention (self-attention variant).

Reference computation (fp32):
    q = x @ Wq.T ; k = x @ Wk.T ; v = x @ Wv.T     (B,N,D) @ (D,D)
    per head (16 heads, dh=64): s = q k^T * dh^-0.5 ; p = softmax(s)
    o = p v ; out = concat(o) @ Wout.T + bout

Sharding: batch*heads across 8 cores. Core c handles batch c//4 and the
4 heads 4*(c%4)..4*(c%4)+3 (a contiguous 256-wide slice of the inner dim).
Each core computes its partial out = o_slice @ Wout[:, slice].T ; the host
sums the 4 partials per batch and adds the bias.

On-device layout trick: everything the PE contracts over must sit on the
SBUF partition axis, so the host ships x and the weight slices already
transposed (xT = x[b].T etc.).  Attention is computed in the transposed
layout sT[j, i] = k_j . q_i so no on-device transposes are needed at all:
  - s-matmul: lhsT = kT[dh, j-tile], rhs = qT[dh, i-block]
  - p = exp(s * scale)  (softmax max-subtraction skipped: |s*scale| < ~3)
  - o-matmul: lhsT = v_aug[j, 65] (col 64 = ones), rhs = p[j, i-block]
    -> oT[d, i] with the softmax denominator in row 64.
  - normalization: recip of the denominator row is broadcast across
    partitions with a K=1 matmul, then fused into the PSUM->SBUF copy.
  - out-projection: lhsT = oT (already transposed!), rhs = WoutT.
"""

import numpy as np

B, N, D = 2, 2048, 1024
H, DH = 16, 64
SCALE = DH**-0.5
NCORES = 8
HLOC = H // 4  # 4 heads per core
DLOC = HLOC * DH  # 256-wide inner slice per core
P = 128

# matmul operand dtype: "f32" (exact, slow), "f32r" (fp32 replicated, fast),
# "bf16" (fast, lower precision)
MM_MODE = "f32r"

_cached = {}


def _build(mm_mode=MM_MODE):
    import concourse.bass as bass
    import concourse.tile as tile
    from concourse import bacc, mybir

    f32 = mybir.dt.float32
    Exp = mybir.ActivationFunctionType.Exp

    if mm_mode == "bf16":
        io_dt = mybir.dt.bfloat16
    else:
        io_dt = f32

    def mm_ap(ap):
        # cast an AP to the matmul compute dtype
        if mm_mode == "f32r":
            return ap.bitcast(mybir.dt.float32r)
        return ap

    nc = bacc.Bacc("TRN2", target_bir_lowering=False, debug=False)

    xT = nc.dram_tensor("xT", [D, N], io_dt, kind="ExternalInput").ap()
    wqT = nc.dram_tensor("wqT", [D, DLOC], io_dt, kind="ExternalInput").ap()
    wkT = nc.dram_tensor("wkT", [D, DLOC], io_dt, kind="ExternalInput").ap()
    wvT = nc.dram_tensor("wvT", [D, DLOC], io_dt, kind="ExternalInput").ap()
    woutT = nc.dram_tensor("woutT", [DLOC, D], io_dt, kind="ExternalInput").ap()
    out = nc.dram_tensor("out", [N, D], f32, kind="ExternalOutput").ap()

    CT = D // P  # 8 c-tiles (contraction tiles for projections)
    NT = N // P  # 16 seq tiles
    DT2 = DLOC // P  # 2 local d-tiles

    with tile.TileContext(nc) as tc:
        with (
            tc.tile_pool(name="big", bufs=1) as big,
            tc.tile_pool(name="stage", bufs=2) as stage,
            tc.tile_pool(name="small", bufs=1) as small,
            tc.tile_pool(name="ps_s", bufs=2, space="PSUM") as ps_s,
            tc.tile_pool(name="ps_o", bufs=4, space="PSUM") as ps_o,
        ):
            # ---- resident SBUF tensors -------------------------------------
            xT_sb = big.tile([P, CT, N], io_dt, tag="xT")
            wqT_sb = big.tile([P, CT, DLOC], io_dt, tag="wqT")
            wkT_sb = big.tile([P, CT, DLOC], io_dt, tag="wkT")
            wvT_sb = big.tile([P, CT, DLOC], io_dt, tag="wvT")
            woutT_sb = big.tile([P, DT2, D], io_dt, tag="woutT")
            qT_sb = big.tile([P, DT2, N], io_dt, tag="qT")
            kT_sb = big.tile([P, DT2, N], io_dt, tag="kT")
            v_sb = big.tile([P, NT, HLOC * (DH + 1)], io_dt, tag="v")
            oT_sb = big.tile([P, DT2, N], io_dt, tag="oT")
            ones_sb = small.tile([1, P], f32, tag="ones")

            nc.vector.memset(ones_sb[:], 1.0)
            for h in range(HLOC):
                # the softmax-denominator ones column of v_aug
                nc.vector.memset(v_sb[:, :, h * (DH + 1) + DH], 1.0)

            # ---- input DMAs ------------------------------------------------
            nc.sync.dma_start(xT_sb[:], xT.rearrange("(c p) n -> p c n", p=P))
            nc.sync.dma_start(wqT_sb[:], wqT.rearrange("(c p) d -> p c d", p=P))
            nc.sync.dma_start(wkT_sb[:], wkT.rearrange("(c p) d -> p c d", p=P))
            nc.sync.dma_start(wvT_sb[:], wvT.rearrange("(c p) d -> p c d", p=P))
            nc.sync.dma_start(woutT_sb[:], woutT.rearrange("(t p) d -> p t d", p=P))

            # ---- projections ----------------------------------------------
            # qT[d, i] = sum_c WqT[c, d] * xT[c, i]; same for kT
            for w_sb, dst in ((wqT_sb, qT_sb), (wkT_sb, kT_sb)):
                for dt_ in range(DT2):
                    for ih in range(2):
                        ps = ps_s.tile([P, 1024], f32, tag="s")
                        for ct in range(CT):
                            for half in range(2):
                                nc.tensor.matmul(
                                    ps[:, half * 512 : half * 512 + 512],
                                    mm_ap(w_sb[:, ct, dt_ * P : dt_ * P + P]),
                                    mm_ap(
                                        dstv := xT_sb[
                                            :,
                                            ct,
                                            ih * 1024 + half * 512 : ih * 1024
                                            + half * 512
                                            + 512,
                                        ]
                                    ),
                                    start=(ct == 0),
                                    stop=(ct == CT - 1),
                                )
                        nc.vector.tensor_copy(
                            dst[:, dt_, ih * 1024 : ih * 1024 + 1024], ps[:]
                        )

            # v[j, d] = sum_c xT[c, j] * WvT[c, d]  (natural layout, + ones col)
            for jt in range(NT):
                psv = ps_s.tile([P, 1024], f32, tag="s")
                for ct in range(CT):
                    nc.tensor.matmul(
                        psv[:, :DLOC],
                        mm_ap(xT_sb[:, ct, jt * P : jt * P + P]),
                        mm_ap(wvT_sb[:, ct, :]),
                        start=(ct == 0),
                        stop=(ct == CT - 1),
                    )
                for h in range(HLOC):
                    nc.vector.tensor_copy(
                        v_sb[:, jt, h * (DH + 1) : h * (DH + 1) + DH],
                        psv[:, h * DH : h * DH + DH],
                    )

            # ---- attention -------------------------------------------------
            for h in range(HLOC):
                hp = h // 2  # which 128-partition block of qT/kT
                ho = (h % 2) * DH  # partition offset within the block
                po_tiles = [
                    ps_o.tile([DH + 1, 512], f32, tag="o", name=f"po_{h}_{i}")
                    for i in range(4)
                ]
                for jt in range(NT):
                    for ih in range(2):
                        pss = ps_s.tile([P, 1024], f32, tag="s")
                        for half in range(2):
                            nc.tensor.matmul(
                                pss[:, half * 512 : half * 512 + 512],
                                mm_ap(kT_sb[ho : ho + DH, hp, jt * P : jt * P + P]),
                                mm_ap(
                                    qT_sb[
                                        ho : ho + DH,
                                        hp,
                                        ih * 1024 + half * 512 : ih * 1024
                                        + half * 512
                                        + 512,
                                    ]
                                ),
                                start=True,
                                stop=True,
                            )
                        p_sb = stage.tile([P, 1024], io_dt, tag="p")
                        nc.scalar.activation(p_sb[:], pss[:], Exp, scale=SCALE)
                        for half in range(2):
                            ib = ih * 2 + half
                            nc.tensor.matmul(
                                po_tiles[ib][:],
                                mm_ap(
                                    v_sb[:, jt, h * (DH + 1) : (h + 1) * (DH + 1)]
                                ),
                                mm_ap(p_sb[:, half * 512 : half * 512 + 512]),
                                start=(jt == 0),
                                stop=(jt == NT - 1),
                            )
                # normalize: oT[d, i] = po[d, i] / po[64, i]
                for ib in range(4):
                    recip = small.tile([1, 512], f32, tag="recip")
                    nc.vector.reciprocal(recip[:], po_tiles[ib][DH : DH + 1, :])
                    bc = ps_s.tile([P, 1024], f32, tag="s")
                    nc.tensor.matmul(
                        bc[:DH, :512],
                        ones_sb[:, :DH],
                        recip[:],
                        start=True,
                        stop=True,
                    )
                    # HW limit: only one PSUM operand per DVE op, so copy
                    # then multiply in place.
                    dst = oT_sb[ho : ho + DH, hp, ib * 512 : ib * 512 + 512]
                    nc.vector.tensor_copy(dst, po_tiles[ib][:DH, :])
                    nc.vector.tensor_mul(dst, dst, bc[:DH, :512])

            # ---- output projection ----------------------------------------
            # out[i, do] = sum_d oT[d, i] * WoutT[d, do]
            for it in range(NT):
                for db in range(2):
                    po = ps_o.tile([P, 512], f32, tag="o")
                    for dt_ in range(DT2):
                        nc.tensor.matmul(
                            po[:],
                            mm_ap(oT_sb[:, dt_, it * P : it * P + P]),
                            mm_ap(woutT_sb[:, dt_, db * 512 : db * 512 + 512]),
                            start=(dt_ == 0),
                            stop=(dt_ == DT2 - 1),
                        )
                    ob = stage.tile([P, 512], f32, tag="ob")
                    nc.vector.tensor_copy(ob[:], po[:])
                    nc.sync.dma_start(
                        out[it * P : it * P + P, db * 512 : db * 512 + 512], ob[:]
                    )

    nc.compile()
    return nc


def get_nc(mm_mode=MM_MODE):
    if mm_mode not in _cached:
        _cached[mm_mode] = _build(mm_mode)
    return _cached[mm_mode]


def make_in_maps(x, Wq, Wk, Wv, Wout, mm_mode=MM_MODE):
    if mm_mode == "bf16":
        import ml_dtypes

        cast = lambda a: np.ascontiguousarray(a, dtype=ml_dtypes.bfloat16)
    else:
        cast = lambda a: np.ascontiguousarray(a, dtype=np.float32)
    in_maps = []
    for c in range(NCORES):
        b = c // 4
        rows = slice((c % 4) * DLOC, (c % 4 + 1) * DLOC)
        in_maps.append(
            {
                "xT": cast(x[b].T),
                "wqT": cast(Wq[rows].T),
                "wkT": cast(Wk[rows].T),
                "wvT": cast(Wv[rows].T),
                "woutT": cast(Wout[:, rows].T),
            }
        )
    return in_maps


def kernel(x, Wq, Wk, Wv, Wout, bout):
    from concourse.bass_utils import run_bass_kernel_spmd

    nc = get_nc()
    in_maps = make_in_maps(x, Wq, Wk, Wv, Wout)
    res = run_bass_kernel_spmd(nc, in_maps, list(range(NCORES)))
    out = np.zeros((B, N, D), np.float32)
    for c in range(NCORES):
        out[c // 4] += res.results[c]["out"]
    out += np.asarray(bout, np.float32)
    return out


# revision 7
# speedup vs baseline: 258.0080x; 258.0080x over previous
"""Trainium2 Bass kernel for CrossAttention (self-attention variant).

Reference computation (fp32):
    q = x @ Wq.T ; k = x @ Wk.T ; v = x @ Wv.T     (B,N,D) @ (D,D)
    per head (16 heads, dh=64): s = q k^T * dh^-0.5 ; p = softmax(s)
    o = p v ; out = concat(o) @ Wout.T + bout

Sharding: batch*heads across 8 cores. Core c handles batch c//4 and the
4 heads 4*(c%4)..4*(c%4)+3 (a contiguous 256-wide slice of the inner dim).
Each core computes its partial out = o_slice @ Wout[:, slice].T ; the host
sums the 4 partials per batch and adds the bias.

On-device layout trick: everything the PE contracts over must sit on the
SBUF partition axis, so the host ships x and the weight slices already
transposed (xT = x[b].T etc.).  Attention is computed in the transposed
layout sT[j, i] = k_j . q_i so no on-device transposes are needed at all:
  - s-matmul: lhsT = kT[dh, j-tile], rhs = qT[dh, i-block]
  - p = exp(s * scale)  (softmax max-subtraction skipped: |s*scale| < ~3)
  - o-matmul: lhsT = v_aug[j, 65] (col 64 = ones), rhs = p[j, i-block]
    -> oT[d, i] with the softmax denominator in row 64.
  - normalization: recip of the denominator row is broadcast across
    partitions with a K=1 matmul, then fused into the PSUM->SBUF copy.
  - out-projection: lhsT = oT (already transposed!), rhs = WoutT.
"""

import numpy as np

B, N, D = 2, 2048, 1024
H, DH = 16, 64
SCALE = DH**-0.5
NCORES = 8
HLOC = H // 4  # 4 heads per core
DLOC = HLOC * DH  # 256-wide inner slice per core
P = 128

# matmul operand dtype: "f32" (exact, slow), "f32r" (fp32 replicated, fast),
# "bf16" (fast, lower precision)
MM_MODE = "f32r"

_cached = {}


def _build(mm_mode=MM_MODE, repeat=1):
    import concourse.bass as bass
    import concourse.tile as tile
    from concourse import bacc, mybir

    f32 = mybir.dt.float32
    Exp = mybir.ActivationFunctionType.Exp

    if mm_mode == "bf16":
        io_dt = mybir.dt.bfloat16
    elif mm_mode == "f32r":
        # fp32r matmul operands must be *produced* as fp32r (the BIR
        # verifier requires rounding at the producer), so the whole
        # activation/weight path is typed float32r; PSUM stays fp32.
        io_dt = mybir.dt.float32r
    else:
        io_dt = f32

    def mm_ap(ap):
        return ap

    nc = bacc.Bacc("TRN2", target_bir_lowering=False, debug=False)

    xT = nc.dram_tensor("xT", [D, N], io_dt, kind="ExternalInput").ap()
    wqT = nc.dram_tensor("wqT", [D, DLOC], io_dt, kind="ExternalInput").ap()
    wkT = nc.dram_tensor("wkT", [D, DLOC], io_dt, kind="ExternalInput").ap()
    wvT = nc.dram_tensor("wvT", [D, DLOC], io_dt, kind="ExternalInput").ap()
    woutT = nc.dram_tensor("woutT", [DLOC, D], io_dt, kind="ExternalInput").ap()
    out = nc.dram_tensor("out", [N, D], f32, kind="ExternalOutput").ap()

    CT = D // P  # 8 c-tiles (contraction tiles for projections)
    NT = N // P  # 16 seq tiles
    DT2 = DLOC // P  # 2 local d-tiles

    with tile.TileContext(nc) as tc:
        with (
            tc.tile_pool(name="big", bufs=1) as big,
            tc.tile_pool(name="stage", bufs=2) as stage,
            tc.tile_pool(name="small", bufs=1) as small,
            tc.tile_pool(name="ps_s", bufs=2, space="PSUM") as ps_s,
            tc.tile_pool(name="ps_o", bufs=4, space="PSUM") as ps_o,
        ):
            ones_sb = small.tile([1, P], f32, tag="ones")
            nc.vector.memset(ones_sb[:], 1.0)

            for rep in range(repeat):
                _emit_iter(
                    nc, tile, mybir, f32, Exp, io_dt, mm_ap, rep,
                    big, stage, small, ps_s, ps_o, ones_sb,
                    xT, wqT, wkT, wvT, woutT, out,
                    CT, NT, DT2,
                )

    nc.compile()
    return nc


def _emit_iter(
    nc, tile, mybir, f32, Exp, io_dt, mm_ap, rep,
    big, stage, small, ps_s, ps_o, ones_sb,
    xT, wqT, wkT, wvT, woutT, out,
    CT, NT, DT2,
):
    # ---- resident SBUF tensors ------------------------------------
    xT_sb = big.tile([P, CT, N], io_dt, tag="xT", name=f"xT_sb_{rep}")
    wqT_sb = big.tile([P, CT, DLOC], io_dt, tag="wqT", name=f"wqT_sb_{rep}")
    wkT_sb = big.tile([P, CT, DLOC], io_dt, tag="wkT", name=f"wkT_sb_{rep}")
    wvT_sb = big.tile([P, CT, DLOC], io_dt, tag="wvT", name=f"wvT_sb_{rep}")
    woutT_sb = big.tile([P, DT2, D], io_dt, tag="woutT", name=f"woutT_sb_{rep}")
    qT_sb = big.tile([P, DT2, N], io_dt, tag="qT", name=f"qT_sb_{rep}")
    kT_sb = big.tile([P, DT2, N], io_dt, tag="kT", name=f"kT_sb_{rep}")
    v_sb = big.tile([P, NT, HLOC * (DH + 1)], io_dt, tag="v", name=f"v_sb_{rep}")
    oT_sb = big.tile([P, DT2, N], io_dt, tag="oT", name=f"oT_sb_{rep}")

    for h in range(HLOC):
        # the softmax-denominator ones column of v_aug. memset can't emit
        # float32r, so write the fp32 bit pattern of 1.0 through uint32.
        col = v_sb[:, :, h * (DH + 1) + DH]
        if io_dt == mybir.dt.float32r:
            nc.vector._memset_packed(col.bitcast(mybir.dt.uint32), 0x3F800000)
        else:
            nc.vector.memset(col, 1.0)

    # ---- input DMAs -----------------------------------------------
    nc.sync.dma_start(xT_sb[:], xT.rearrange("(c p) n -> p c n", p=P))
    nc.sync.dma_start(wqT_sb[:], wqT.rearrange("(c p) d -> p c d", p=P))
    nc.sync.dma_start(wkT_sb[:], wkT.rearrange("(c p) d -> p c d", p=P))
    nc.sync.dma_start(wvT_sb[:], wvT.rearrange("(c p) d -> p c d", p=P))
    nc.sync.dma_start(woutT_sb[:], woutT.rearrange("(t p) d -> p t d", p=P))

    # ---- projections ----------------------------------------------
    # qT[d, i] = sum_c WqT[c, d] * xT[c, i]; same for kT
    for w_sb, dst in ((wqT_sb, qT_sb), (wkT_sb, kT_sb)):
        for dt_ in range(DT2):
            for ih in range(2):
                ps = ps_s.tile([P, 1024], f32, tag="s", name=f"psqk_{rep}_{dt_}_{ih}")
                for ct in range(CT):
                    for half in range(2):
                        nc.tensor.matmul(
                            ps[:, half * 512 : half * 512 + 512],
                            mm_ap(w_sb[:, ct, dt_ * P : dt_ * P + P]),
                            mm_ap(
                                xT_sb[
                                    :,
                                    ct,
                                    ih * 1024
                                    + half * 512 : ih * 1024
                                    + half * 512
                                    + 512,
                                ]
                            ),
                            start=(ct == 0),
                            stop=(ct == CT - 1),
                        )
                nc.vector.tensor_copy(
                    dst[:, dt_, ih * 1024 : ih * 1024 + 1024], ps[:]
                )

    # v[j, d] = sum_c xT[c, j] * WvT[c, d]  (natural layout, + ones col)
    for jt in range(NT):
        psv = ps_s.tile([P, 1024], f32, tag="s", name=f"psv_{rep}_{jt}")
        for ct in range(CT):
            nc.tensor.matmul(
                psv[:, :DLOC],
                mm_ap(xT_sb[:, ct, jt * P : jt * P + P]),
                mm_ap(wvT_sb[:, ct, :]),
                start=(ct == 0),
                stop=(ct == CT - 1),
            )
        for h in range(HLOC):
            nc.vector.tensor_copy(
                v_sb[:, jt, h * (DH + 1) : h * (DH + 1) + DH],
                psv[:, h * DH : h * DH + DH],
            )

    # ---- attention -------------------------------------------------
    for h in range(HLOC):
        hp = h // 2  # which 128-partition block of qT/kT
        ho = (h % 2) * DH  # partition offset within the block
        po_tiles = [
            ps_o.tile([DH + 1, 512], f32, tag="o", name=f"po_{rep}_{h}_{i}")
            for i in range(4)
        ]
        for jt in range(NT):
            for ih in range(2):
                pss = ps_s.tile([P, 1024], f32, tag="s", name=f"pss_{rep}_{h}_{jt}_{ih}")
                for half in range(2):
                    nc.tensor.matmul(
                        pss[:, half * 512 : half * 512 + 512],
                        mm_ap(kT_sb[ho : ho + DH, hp, jt * P : jt * P + P]),
                        mm_ap(
                            qT_sb[
                                ho : ho + DH,
                                hp,
                                ih * 1024
                                + half * 512 : ih * 1024
                                + half * 512
                                + 512,
                            ]
                        ),
                        start=True,
                        stop=True,
                    )
                p_sb = stage.tile([P, 1024], io_dt, tag="p", name=f"p_sb_{rep}_{h}_{jt}_{ih}")
                nc.scalar.activation(p_sb[:], pss[:], Exp, scale=SCALE)
                for half in range(2):
                    ib = ih * 2 + half
                    nc.tensor.matmul(
                        po_tiles[ib][:],
                        mm_ap(v_sb[:, jt, h * (DH + 1) : (h + 1) * (DH + 1)]),
                        mm_ap(p_sb[:, half * 512 : half * 512 + 512]),
                        start=(jt == 0),
                        stop=(jt == NT - 1),
                    )
        # normalize: oT[d, i] = po[d, i] / po[64, i]
        for ib in range(4):
            recip = small.tile([1, 512], f32, tag="recip", name=f"recip_{rep}_{h}_{ib}")
            nc.vector.reciprocal(recip[:], po_tiles[ib][DH : DH + 1, :])
            bc = ps_s.tile([P, 1024], f32, tag="s", name=f"bc_{rep}_{h}_{ib}")
            nc.tensor.matmul(
                bc[:DH, :512], ones_sb[:, :DH], recip[:], start=True, stop=True
            )
            # HW limit: only one PSUM operand per DVE op, so copy then
            # multiply in place.
            dst = oT_sb[ho : ho + DH, hp, ib * 512 : ib * 512 + 512]
            nc.vector.tensor_copy(dst, po_tiles[ib][:DH, :])
            nc.vector.tensor_mul(dst, dst, bc[:DH, :512])

    # ---- output projection ----------------------------------------
    # out[i, do] = sum_d oT[d, i] * WoutT[d, do]
    for it in range(NT):
        for db in range(2):
            po = ps_o.tile([P, 512], f32, tag="o", name=f"pso_{rep}_{it}_{db}")
            for dt_ in range(DT2):
                nc.tensor.matmul(
                    po[:],
                    mm_ap(oT_sb[:, dt_, it * P : it * P + P]),
                    mm_ap(woutT_sb[:, dt_, db * 512 : db * 512 + 512]),
                    start=(dt_ == 0),
                    stop=(dt_ == DT2 - 1),
                )
            ob = stage.tile([P, 512], f32, tag="ob", name=f"ob_{rep}_{it}_{db}")
            nc.vector.tensor_copy(ob[:], po[:])
            nc.sync.dma_start(
                out[it * P : it * P + P, db * 512 : db * 512 + 512], ob[:]
            )


def get_nc(mm_mode=MM_MODE, repeat=1):
    key = (mm_mode, repeat)
    if key not in _cached:
        _cached[key] = _build(mm_mode, repeat)
    return _cached[key]


def make_in_maps(x, Wq, Wk, Wv, Wout, mm_mode=MM_MODE):
    if mm_mode == "bf16":
        import ml_dtypes

        cast = lambda a: np.ascontiguousarray(np.asarray(a), dtype=ml_dtypes.bfloat16)
    else:
        cast = lambda a: np.ascontiguousarray(np.asarray(a), dtype=np.float32)
    x, Wq, Wk, Wv, Wout = (np.asarray(a) for a in (x, Wq, Wk, Wv, Wout))
    in_maps = []
    for c in range(NCORES):
        b = c // 4
        rows = slice((c % 4) * DLOC, (c % 4 + 1) * DLOC)
        in_maps.append(
            {
                "xT": cast(x[b].T),
                "wqT": cast(Wq[rows].T),
                "wkT": cast(Wk[rows].T),
                "wvT": cast(Wv[rows].T),
                "woutT": cast(Wout[:, rows].T),
            }
        )
    return in_maps


def kernel(x, Wq, Wk, Wv, Wout, bout):
    from concourse.bass_utils import run_bass_kernel_spmd

    nc = get_nc()
    in_maps = make_in_maps(x, Wq, Wk, Wv, Wout)
    res = run_bass_kernel_spmd(nc, in_maps, list(range(NCORES)))
    out = np.zeros((B, N, D), np.float32)
    for c in range(NCORES):
        out[c // 4] += res.results[c]["out"]
    out += np.asarray(bout, np.float32)
    return out
